# revision 8
# baseline (speedup 1.0000x reference)
"""Trainium2 Bass kernel for the adaptive semantic-scal loss (segment_reduce).

Self-contained: hardcodes shapes/sharding for
  pred [2,17,200,200,16] f32, ssc_target [2,200,200,16] int, f1_list [17] f32.

Strategy (8 NeuronCores, data-parallel over voxels; 160k voxels/core laid
out as 128 partitions x 1250 voxels, slab-major / class-major within each
partition; every 125-voxel chunk gets a leading "gap" column):

  device, per core: 5 pipelined slabs of 2 chunks each.
    ACT: E = exp(pred) per slab (the hard floor: ~19us at 1 elem/cyc).
    DVE: onehot prebuilt for the whole tile (overlaps the DMA ramp),
         per slab: class-tree-sum -> S, fast reciprocal -> W (bf16),
         R = E*W in place, gap columns of R set to 1.
    PE:  per class c<16, per chunk: psum[c] += OH_chunk^T @ R_chunk into a
         single PSUM set (10-chunk accumulation groups); the gap columns
         make row 0 = sum_p partials and col 0 = count partials, the
         diagonal holds nominator partials.
    class 16 nominator: one fused STT (onehot*R + free-dim accum) per slab,
         written straight into the output tile.
    extraction (once): mask-mult + X-reduce of the PSUM set -> per-position
         nominator/sum_p partials; strided copy of col 0 -> count partials.
  device output: [128, 64] f32 of partial sums per core - NO collective,
  NO on-device epilogue.

  host: gather 8x[128,64], sum partials (cores+partitions), close classes
  16/sum_p/count via softmax identities, run the 17-element scalar loss
  epilogue in numpy.
"""

import sys

for _p in ("/opt/trn_rl_repo",):
    if _p not in sys.path:
        sys.path.append(_p)

import numpy as np
import ml_dtypes

import concourse.bacc as bacc
import concourse.tile as tile
import concourse.mybir as mybir
from concourse.bass_utils import run_bass_kernel_spmd

F32 = mybir.dt.float32
BF16 = mybir.dt.bfloat16
ALU = mybir.AluOpType
ACTF = mybir.ActivationFunctionType

N_CORES = 8
P = 128          # partitions
C = 17           # classes
KV = 1250        # real voxels per partition per core (128*1250*8 = 1.28M)
W = 125          # data voxels per matmul chunk
WP = W + 1       # chunk width incl. leading ones-gap column
NCH = 10         # chunks per partition
KVP = NCH * WP   # padded voxels per partition (1260)
NSLAB = 5        # pipeline slabs
CPS = NCH // NSLAB   # chunks per slab (2)
SW = CPS * WP        # slab width (252)

BETA = 0.95
ALPHA = 5.0
WPC = 3.0
NTOT = float(N_CORES * P * KV)  # all targets are valid (0..16)


def _build():
    nc = bacc.Bacc("TRN2", target_bir_lowering=False, debug=False,
                   num_devices=N_CORES)
    pred_d = nc.dram_tensor("pred", [P, NSLAB * C * SW], BF16,
                            kind="ExternalInput")
    tgt_d = nc.dram_tensor("tgt", [P, KVP], BF16, kind="ExternalInput")
    out_d = nc.dram_tensor("out", [P, 64], F32, kind="ExternalOutput")

    # extraction mask over a [126, 256] two-class bank view: per half,
    # diag (1..125) -> nominator cells, row 0 (cols>=1) -> sum_p cells;
    # [*, half*128] stays 0 (gap x gap junk)
    m2 = np.zeros((128, 256), np.float32)
    for half in range(2):
        for k in range(1, WP):
            m2[k, half * 128 + k] = 1.0
            m2[0, half * 128 + k] = 1.0

    with tile.TileContext(nc) as tc:
        with (
            tc.tile_pool(name="pred", bufs=1) as pk,
            tc.tile_pool(name="work", bufs=1) as pw,
            tc.tile_pool(name="small", bufs=2) as ps,
            tc.tile_pool(name="persist", bufs=1) as pa,
            tc.tile_pool(name="psum", bufs=1, space="PSUM") as pp,
        ):
            tgt_sb = pa.tile([P, NSLAB, SW], BF16)
            nc.sync.dma_start(
                out=tgt_sb[:, :, :].rearrange("p s k -> p (s k)"),
                in_=tgt_d[:, :])
            mask2_d = nc.inline_tensor(m2.astype(ml_dtypes.bfloat16),
                                       name="mask2")
            mask2 = pa.tile([128, 256], BF16)
            nc.sync.dma_start(out=mask2[:, :], in_=mask2_d[:, :])

            pred_sb = pk.tile([P, NSLAB, C, SW], BF16)
            for s in range(NSLAB):
                nc.sync.dma_start(
                    out=pred_sb[:, s, :, :].rearrange("p c k -> p (c k)"),
                    in_=pred_d[:, s * C * SW:(s + 1) * C * SW])

            ER = pw.tile([P, NSLAB, C, SW], BF16)      # E, then R in place
            OH = pa.tile([P, 16, NSLAB, SW], BF16)     # onehot, class-major
            out_sb = pa.tile([P, 64], F32)

            # ---- ACT: exp per slab ----------------------------------
            for s in range(NSLAB):
                nc.scalar.activation(
                    ER[:, s, :, :].rearrange("p c k -> p (c k)"),
                    pred_sb[:, s, :, :].rearrange("p c k -> p (c k)"),
                    ACTF.Exp)

            # ---- DVE queue ------------------------------------------
            # onehot prebuild for the whole tile (depends only on tgt,
            # which lands first -> overlaps the pred DMA / ACT ramp)
            tgt_flat = tgt_sb[:, :, :].rearrange("p s k -> p (s k)")
            for c in range(16):
                nc.vector.tensor_scalar(
                    OH[:, c, :, :].rearrange("p s k -> p (s k)"),
                    tgt_flat, float(c), None, ALU.is_equal)
            # gap columns of OH -> 1 (row-0 sum_p trick)
            for s in range(NSLAB):
                nc.vector.memset(
                    OH[:, :, s, :].rearrange("p c (g k) -> p c g k",
                                             g=CPS)[:, :, :, 0], 1.0)

            def emit_slab_dve(s):
                T8 = ps.tile([P, 8, SW], BF16, name="t8_%d" % s, tag="t8",
                             bufs=2)
                S = ps.tile([P, SW], F32, name="s_%d" % s, tag="s", bufs=2)
                Wf = ps.tile([P, SW], F32, name="wf_%d" % s, tag="wf", bufs=2)
                Wb = ps.tile([P, SW], BF16, name="w_%d" % s, tag="w", bufs=2)
                dmp = ps.tile([P, SW], BF16, name="d_%d" % s, tag="d", bufs=2)
                e = ER[:, s]
                nc.vector.tensor_add(T8[:, :, :], e[:, 0:8, :], e[:, 8:16, :])
                nc.vector.tensor_add(T8[:, 0:4, :], T8[:, 0:4, :],
                                     T8[:, 4:8, :])
                nc.vector.tensor_add(T8[:, 0:2, :], T8[:, 0:2, :],
                                     T8[:, 2:4, :])
                nc.vector.tensor_add(T8[:, 0, :], T8[:, 0, :], T8[:, 1, :])
                nc.vector.tensor_add(S[:, :], T8[:, 0, :], e[:, 16, :])
                nc.vector.reciprocal_approx_fast(Wf[:, :], S[:, :])
                nc.vector.tensor_copy(Wb[:, :], Wf[:, :])
                wb = Wb[:, :].rearrange("p (a k) -> p a k", a=1) \
                    .to_broadcast((P, C, SW))
                nc.vector.tensor_tensor(e[:, :, :], e[:, :, :], wb,
                                        op=ALU.mult)
                # gap columns of R -> 1 (col-0 count trick), classes 0..15
                nc.vector.memset(
                    e[:, 0:16, :].rearrange("p c (g k) -> p c g k",
                                            g=CPS)[:, :, :, 0], 1.0)
                # class-16 nominator partial straight into the output tile
                nc.vector.scalar_tensor_tensor(
                    out=dmp[:, :], in0=tgt_sb[:, s, :], scalar=16.0,
                    in1=e[:, 16, :], op0=ALU.is_equal, op1=ALU.mult,
                    accum_out=out_sb[:, 32 + s:33 + s])

            for s in range(NSLAB):
                emit_slab_dve(s)

            # ---- PE: 8 banks, one accumulation chain per bank -------
            # classes c and c+8 share bank c at column halves 0/128; one
            # start per bank (clears the bank's has_written), the sibling
            # class's first write overwrites (bits clear), rest accumulate.
            pnom = pp.tile([128, 8, 512], F32)
            for s in range(NSLAB):
                for h in range(CPS):
                    g = s * CPS + h
                    for c in range(16):
                        bank, colb = c % 8, 128 * (c // 8)
                        nc.tensor.matmul(
                            pnom[0:WP, bank, colb:colb + WP],
                            OH[:, c, s, h * WP:(h + 1) * WP],
                            ER[:, s, c, h * WP:(h + 1) * WP],
                            start=(g == 0 and c < 8),
                            stop=(g == NCH - 1 and c >= 8),
                            skip_group_check=True)

            # ---- extraction (once) ----------------------------------
            # device class order in the 16 output cols: j = 2*bank + half,
            # i.e. class = (j%2)*8 + j//2 (host unpermutes)
            nd = pw.tile([128, 8, 256], BF16)
            m2b = mask2[0:WP, :].rearrange("p (a k) -> p a k", a=1) \
                .to_broadcast((WP, 8, 256))
            nc.vector.tensor_tensor(nd[0:WP, :, :], pnom[0:WP, :, 0:256],
                                    m2b, op=ALU.mult)
            # rows 1..125 -> nominator partials, row 0 -> sum_p partials
            nc.vector.tensor_reduce(
                out_sb[0:WP, 16:32],
                nd[0:WP, :, :].rearrange("p b (h k) -> p (b h) k", h=2),
                axis=mybir.AxisListType.X, op=ALU.add)
            # count partials: col 0 of each half-region (row 0 junk,
            # host skips it)
            nc.vector.tensor_copy(
                out_sb[0:WP, 0:16],
                pnom[0:WP, :, 0:256].rearrange("p b (h k) -> p b h k",
                                               h=2)[:, :, :, 0])
            nc.sync.dma_start(out=out_d[:, :], in_=out_sb[:, :])

    nc.compile()
    return nc


_NC_CACHE = None


def _get_nc():
    global _NC_CACHE
    if _NC_CACHE is None:
        _NC_CACHE = _build()
    return _NC_CACHE


def _shard_inputs(pred, ssc_target, f1_list=None):
    pred = np.asarray(pred, dtype=np.float32)
    tgt = np.asarray(ssc_target)

    nvox = N_CORES * P * KV
    assert nvox == pred.size // C
    # voxel-major [v, c], then block: [core, p, c, kv]
    pv = np.ascontiguousarray(
        pred.reshape(2, C, -1).transpose(0, 2, 1).reshape(nvox, C)
        .reshape(N_CORES, P, KV, C).transpose(0, 1, 3, 2))
    tv = tgt.reshape(nvox).reshape(N_CORES, P, KV)
    # pad: each 125-voxel chunk gets a leading gap column
    # (pred=0 -> E=1; tgt=255 -> onehot=0)
    pp_ = np.zeros((N_CORES, P, C, NCH, WP), np.float32)
    pp_[..., 1:] = pv.reshape(N_CORES, P, C, NCH, W)
    # slab-major layout: [core, p, slab, c, chunk-in-slab cols]
    pp_ = pp_.reshape(N_CORES, P, C, NSLAB, CPS * WP).transpose(0, 1, 3, 2, 4)
    pf = np.ascontiguousarray(pp_.reshape(N_CORES, P, NSLAB * C * SW)) \
        .astype(ml_dtypes.bfloat16)
    tp = np.full((N_CORES, P, NCH, WP), 255.0, np.float32)
    tp[..., 1:] = tv.reshape(N_CORES, P, NCH, W)
    tp = tp.reshape(N_CORES, P, KVP).astype(ml_dtypes.bfloat16)
    return [{"pred": pf[i], "tgt": tp[i]} for i in range(N_CORES)]


def _postprocess(outs, f1_list):
    """outs: list of per-core [128, 64] f32 partial tiles -> scalar loss."""
    a = np.asarray(outs, dtype=np.float64)          # [cores, 128, 64]
    count = np.zeros(C)
    sum_p = np.zeros(C)
    nom = np.zeros(C)
    # device col j = 2*(c%8) + (c//8) for class c
    perm = np.array([2 * (c % 8) + (c // 8) for c in range(16)])
    count[:16] = a[:, 1:WP, 0:16].sum(axis=(0, 1))[perm]
    nom[:16] = a[:, 1:WP, 16:32].sum(axis=(0, 1))[perm]
    sum_p[:16] = a[:, 0, 16:32].sum(axis=0)[perm]
    nom[16] = a[:, :, 32:32 + NSLAB].sum()
    count[16] = NTOT - count[:16].sum()
    sum_p[16] = NTOT - sum_p[:16].sum()
    n_mask = NTOT

    f1_list = np.asarray(f1_list, dtype=np.float64)
    has = count > 0
    pm = sum_p > 0
    precision = np.where(pm, nom / np.where(pm, sum_p, 1.0), 0.0)
    recall = np.where(has, nom / np.where(has, count, 1.0), 0.0)
    neg = n_mask - count
    spec_num = (n_mask - sum_p) - (count - nom)
    nmp = neg > 0
    specificity = np.where(nmp, spec_num / np.where(nmp, neg, 1.0), 0.0)

    def bce(x):
        return np.minimum(-np.log(np.maximum(x, 1e-38)), 100.0)

    loss_list = np.where(
        has,
        np.where(pm, bce(precision), 0.0) + bce(recall)
        + np.where(nmp, bce(specificity), 0.0),
        0.0)

    denom = precision + recall
    f1 = np.where(denom > 0, 2.0 * precision * recall
                  / np.where(denom > 0, denom, 1.0), 0.0)
    cur_f1 = np.where(has, f1, 0.0)
    new_f1 = BETA * f1_list + (1.0 - BETA) * cur_f1

    cnt = has.sum()
    sel = loss_list != 0
    logits = np.where(sel, ALPHA * (1.0 - new_f1), -np.inf)
    mx = logits.max()
    ex = np.exp(logits - mx)
    sm = ex / ex.sum()
    weighted = loss_list * (1.0 + WPC * cnt * sm)
    loss = weighted.sum() / (cnt * (1.0 + WPC))
    return np.float32(loss)


def kernel(pred, ssc_target, f1_list):
    nc = _get_nc()
    in_maps = _shard_inputs(pred, ssc_target)
    res = run_bass_kernel_spmd(nc, in_maps, core_ids=list(range(N_CORES)))
    outs = [np.asarray(r["out"], dtype=np.float32) for r in res.results]
    return _postprocess(outs, f1_list).reshape(())


if __name__ == "__main__":
    rng = np.random.default_rng(0)
    pred = rng.standard_normal((2, C, 200, 200, 16), dtype=np.float32)
    tgt = rng.integers(0, C, size=(2, 200, 200, 16)).astype(np.int64)
    f1l = np.zeros((C,), np.float32)
    print(kernel(pred, tgt, f1l))


# revision 10
# speedup vs baseline: 1.1949x; 1.1949x over previous
"""Trainium2 Bass kernel for the adaptive semantic-scal loss (segment_reduce).

Self-contained: hardcodes shapes/sharding for
  pred [2,17,200,200,16] f32, ssc_target [2,200,200,16] int, f1_list [17] f32.

Strategy (8 NeuronCores, data-parallel over voxels; 160k voxels/core laid
out as 128 partitions x 1250 voxels, slab-major / class-major within each
partition; every 125-voxel chunk gets a leading "gap" column):

  device, per core: pipelined slabs (3+2+2+2+1 chunks of 126 cols).
    pred ships as fp8_e4m3 (halves HBM traffic; noise averages out in the
    75k-element class sums).
    ACT: E = exp(pred) per slab (the hard floor: ~19us at 1 elem/cyc).
    DVE: onehot prebuilt for the whole tile (overlaps the DMA ramp),
         per slab: class-tree-sum -> S, fast reciprocal -> W (bf16),
         R = E*W in place, gap columns of R set to 1.
    PE:  per class c<16, per chunk: psum += OH_chunk^T @ R_chunk; classes
         c and c+8 share PSUM bank c as ONE accumulation chain (a start
         clears the whole bank's has_written bits, so only one start per
         bank; the sibling's first write lands on cleared bits and
         overwrites). Gap columns make row 0 = sum_p partials and col 0 =
         count partials; the diagonal holds nominator partials.
    class 16 nominator: one fused STT (onehot*R + free-dim accum) per slab,
         written straight into the output tile.
    extraction (once): mask-mult + X-reduce (bf16, 4x mode) of the PSUM
         set -> per-position nominator/sum_p partials; strided copy of
         col 0 -> count partials.
  device output: [128, 64] f32 of partial sums per core - NO collective,
  NO on-device epilogue.

  host: gather 8x[128,64], sum partials (cores+partitions), close classes
  16/sum_p/count via softmax identities, run the 17-element scalar loss
  epilogue in numpy.
"""

import sys

for _p in ("/opt/trn_rl_repo",):
    if _p not in sys.path:
        sys.path.append(_p)

import numpy as np
import ml_dtypes

import concourse.bacc as bacc
import concourse.tile as tile
import concourse.mybir as mybir
from concourse.bass_utils import run_bass_kernel_spmd

F32 = mybir.dt.float32
BF16 = mybir.dt.bfloat16
FP8 = mybir.dt.float8e4
ALU = mybir.AluOpType
ACTF = mybir.ActivationFunctionType

N_CORES = 8
P = 128          # partitions
C = 17           # classes
KV = 1250        # real voxels per partition per core (128*1250*8 = 1.28M)
W = 125          # data voxels per matmul chunk
WP = W + 1       # chunk width incl. leading ones-gap column
NCH = 10         # chunks per partition
KVP = NCH * WP   # padded voxels per partition (1260)
CPSL = [3, 2, 2, 2, 1]            # chunks per slab
NSLAB = len(CPSL)
COFF = [sum(CPSL[:i]) for i in range(NSLAB)]   # chunk offset per slab

BETA = 0.95
ALPHA = 5.0
WPC = 3.0
NTOT = float(N_CORES * P * KV)  # all targets are valid (0..16)


def _build():
    nc = bacc.Bacc("TRN2", target_bir_lowering=False, debug=False,
                   num_devices=N_CORES)
    pred_d = nc.dram_tensor("pred", [P, C * KVP], FP8, kind="ExternalInput")
    tgt_d = nc.dram_tensor("tgt", [P, KVP], BF16, kind="ExternalInput")
    out_d = nc.dram_tensor("out", [P, 64], F32, kind="ExternalOutput")

    # extraction mask over a [126, 256] two-class bank view: per half,
    # diag (1..125) -> nominator cells, row 0 (cols>=1) -> sum_p cells;
    # [*, half*128] stays 0 (gap x gap junk)
    m2 = np.zeros((128, 256), np.float32)
    for half in range(2):
        for k in range(1, WP):
            m2[k, half * 128 + k] = 1.0
            m2[0, half * 128 + k] = 1.0

    with tile.TileContext(nc) as tc:
        with (
            tc.tile_pool(name="pred", bufs=1) as pk,
            tc.tile_pool(name="work", bufs=1) as pw,
            tc.tile_pool(name="small", bufs=2) as ps,
            tc.tile_pool(name="persist", bufs=1) as pa,
            tc.tile_pool(name="psum", bufs=1, space="PSUM") as pp,
        ):
            tgt_sb = pa.tile([P, KVP], BF16)
            nc.sync.dma_start(out=tgt_sb[:, :], in_=tgt_d[:, :])

            pred_sb = pk.tile([P, C, KVP], FP8)   # slab-major runs
            for s in range(NSLAB):
                a = C * WP * COFF[s]
                b = C * WP * (COFF[s] + CPSL[s])
                nc.sync.dma_start(
                    out=pred_sb[:, :, :].rearrange("p c k -> p (c k)")[:, a:b],
                    in_=pred_d[:, a:b])

            mask2_d = nc.inline_tensor(m2.astype(ml_dtypes.bfloat16),
                                       name="mask2")
            mask2 = pa.tile([128, 256], BF16)
            nc.sync.dma_start(out=mask2[:, :], in_=mask2_d[:, :])

            # slab views: [P, C, w] starting at slab-major flat offsets
            def slab_view(tile_, s, dt_w=1):
                flat = tile_[:, :, :].rearrange("p c k -> p (c k)")
                a = C * WP * COFF[s]
                w = WP * CPSL[s]
                return flat[:, a:a + C * w].rearrange(
                    "p (c k) -> p c k", c=C)

            ER = pw.tile([P, C, KVP], BF16)        # E, then R in place
            OH = pa.tile([P, 16, KVP], BF16)       # onehot, chunk-major
            out_sb = pa.tile([P, 64], F32)

            # ---- ACT: exp per slab ----------------------------------
            for s in range(NSLAB):
                a = C * WP * COFF[s]
                b = C * WP * (COFF[s] + CPSL[s])
                nc.scalar.activation(
                    ER[:, :, :].rearrange("p c k -> p (c k)")[:, a:b],
                    pred_sb[:, :, :].rearrange("p c k -> p (c k)")[:, a:b],
                    ACTF.Exp)

            # ---- DVE queue ------------------------------------------
            # onehot prebuild for the whole tile (depends only on tgt,
            # which lands first -> overlaps the pred DMA / ACT ramp)
            for c in range(16):
                nc.vector.tensor_scalar(OH[:, c, :], tgt_sb[:, :],
                                        float(c), None, ALU.is_equal)
            # gap columns of OH -> 1 (row-0 sum_p trick)
            nc.vector.memset(
                OH[:, :, :].rearrange("p c (g k) -> p c g k",
                                      g=NCH)[:, :, :, 0], 1.0)

            def emit_slab_dve(s):
                w = WP * CPSL[s]
                T8 = ps.tile([P, 8, WP * 3], BF16, name="t8_%d" % s,
                             tag="t8", bufs=2)
                S = ps.tile([P, WP * 3], F32, name="s_%d" % s, tag="s",
                            bufs=2)
                Wf = ps.tile([P, WP * 3], F32, name="wf_%d" % s, tag="wf",
                             bufs=2)
                Wb = ps.tile([P, WP * 3], BF16, name="w_%d" % s, tag="w",
                             bufs=2)
                dmp = ps.tile([P, WP * 3], BF16, name="d_%d" % s, tag="d",
                              bufs=2)
                e = slab_view(ER, s)
                nc.vector.tensor_add(T8[:, :, 0:w], e[:, 0:8, :],
                                     e[:, 8:16, :])
                nc.vector.tensor_add(T8[:, 0:4, 0:w], T8[:, 0:4, 0:w],
                                     T8[:, 4:8, 0:w])
                nc.vector.tensor_add(T8[:, 0:2, 0:w], T8[:, 0:2, 0:w],
                                     T8[:, 2:4, 0:w])
                nc.vector.tensor_add(T8[:, 0, 0:w], T8[:, 0, 0:w],
                                     T8[:, 1, 0:w])
                nc.vector.tensor_add(S[:, 0:w], T8[:, 0, 0:w], e[:, 16, :])
                nc.vector.reciprocal_approx_fast(Wf[:, 0:w], S[:, 0:w])
                nc.vector.tensor_copy(Wb[:, 0:w], Wf[:, 0:w])
                wb = Wb[:, 0:w].rearrange("p (a k) -> p a k", a=1) \
                    .to_broadcast((P, C, w))
                nc.vector.tensor_tensor(e[:, :, :], e[:, :, :], wb,
                                        op=ALU.mult)
                # gap columns of R -> 1 (col-0 count trick), classes 0..15
                nc.vector.memset(
                    e[:, 0:16, :].rearrange("p c (g k) -> p c g k",
                                            g=CPSL[s])[:, :, :, 0], 1.0)
                # class-16 nominator partial straight into the output tile
                tg = tgt_sb[:, WP * COFF[s]:WP * COFF[s] + w]
                nc.vector.scalar_tensor_tensor(
                    out=dmp[:, 0:w], in0=tg, scalar=16.0,
                    in1=e[:, 16, :], op0=ALU.is_equal, op1=ALU.mult,
                    accum_out=out_sb[:, 32 + s:33 + s])

            for s in range(NSLAB):
                emit_slab_dve(s)

            # ---- PE: 8 banks, one accumulation chain per bank -------
            pnom = pp.tile([128, 8, 512], F32)
            for s in range(NSLAB):
                e = slab_view(ER, s)
                for h in range(CPSL[s]):
                    g = COFF[s] + h
                    for c in range(16):
                        bank, colb = c % 8, 128 * (c // 8)
                        nc.tensor.matmul(
                            pnom[0:WP, bank, colb:colb + WP],
                            OH[:, c, g * WP:(g + 1) * WP],
                            e[:, c, h * WP:(h + 1) * WP],
                            start=(g == 0 and c < 8),
                            stop=(g == NCH - 1 and c >= 8),
                            skip_group_check=True)

            # ---- extraction (once) ----------------------------------
            # device class order in the 16 output cols: j = 2*bank + half,
            # i.e. class = (j%2)*8 + j//2 (host unpermutes)
            nd = pw.tile([128, 8, 256], BF16)
            Vt = ps.tile([128, 16], BF16, name="vt", tag="vt")
            m2b = mask2[0:WP, :].rearrange("p (a k) -> p a k", a=1) \
                .to_broadcast((WP, 8, 256))
            nc.vector.tensor_tensor(nd[0:WP, :, :], pnom[0:WP, :, 0:256],
                                    m2b, op=ALU.mult)
            # rows 1..125 -> nominator partials, row 0 -> sum_p partials
            # (bf16 accumulation is fine: ~16-element sums of ~O(1) values,
            # host averages the noise across 126x8 partials)
            with nc.allow_low_precision(reason="bf16 partials, 4x mode"):
                nc.vector.tensor_reduce(
                    Vt[0:WP, :],
                    nd[0:WP, :, :].rearrange("p b (h k) -> p (b h) k", h=2),
                    axis=mybir.AxisListType.X, op=ALU.add)
            nc.vector.tensor_copy(out_sb[0:WP, 16:32], Vt[0:WP, :])
            # count partials: col 0 of each half-region (row 0 junk,
            # host skips it)
            nc.vector.tensor_copy(
                out_sb[0:WP, 0:16],
                pnom[0:WP, :, 0:256].rearrange("p b (h k) -> p b h k",
                                               h=2)[:, :, :, 0])
            nc.sync.dma_start(out=out_d[:, :], in_=out_sb[:, :])

    nc.compile()
    return nc


_NC_CACHE = None


def _get_nc():
    global _NC_CACHE
    if _NC_CACHE is None:
        _NC_CACHE = _build()
    return _NC_CACHE


def _shard_inputs(pred, ssc_target, f1_list=None):
    pred = np.asarray(pred, dtype=np.float32)
    tgt = np.asarray(ssc_target)

    nvox = N_CORES * P * KV
    assert nvox == pred.size // C
    # voxel-major [v, c], then block: [core, p, c, kv]
    pv = np.ascontiguousarray(
        pred.reshape(2, C, -1).transpose(0, 2, 1).reshape(nvox, C)
        .reshape(N_CORES, P, KV, C).transpose(0, 1, 3, 2))
    tv = tgt.reshape(nvox).reshape(N_CORES, P, KV)
    # pad: each 125-voxel chunk gets a leading gap column
    # (pred=0 -> E=1; tgt=255 -> onehot=0)
    pp_ = np.zeros((N_CORES, P, C, NCH, WP), np.float32)
    pp_[..., 1:] = pv.reshape(N_CORES, P, C, NCH, W)
    # slab-major layout: [core, p, slab, c, slab cols]
    pp_ = pp_.reshape(N_CORES, P, C, KVP)
    parts = []
    for s in range(NSLAB):
        a = WP * COFF[s]
        b = a + WP * CPSL[s]
        parts.append(pp_[:, :, :, a:b].reshape(N_CORES, P, -1))
    pf = np.ascontiguousarray(np.concatenate(parts, axis=2)) \
        .astype(ml_dtypes.float8_e4m3)
    tp = np.full((N_CORES, P, NCH, WP), 255.0, np.float32)
    tp[..., 1:] = tv.reshape(N_CORES, P, NCH, W)
    tp = tp.reshape(N_CORES, P, KVP).astype(ml_dtypes.bfloat16)
    return [{"pred": pf[i], "tgt": tp[i]} for i in range(N_CORES)]


def _postprocess(outs, f1_list):
    """outs: list of per-core [128, 64] f32 partial tiles -> scalar loss."""
    a = np.asarray(outs, dtype=np.float64)          # [cores, 128, 64]
    count = np.zeros(C)
    sum_p = np.zeros(C)
    nom = np.zeros(C)
    # device col j = 2*(c%8) + (c//8) for class c
    perm = np.array([2 * (c % 8) + (c // 8) for c in range(16)])
    count[:16] = a[:, 1:WP, 0:16].sum(axis=(0, 1))[perm]
    nom[:16] = a[:, 1:WP, 16:32].sum(axis=(0, 1))[perm]
    sum_p[:16] = a[:, 0, 16:32].sum(axis=0)[perm]
    nom[16] = a[:, :, 32:32 + NSLAB].sum()
    count[16] = NTOT - count[:16].sum()
    sum_p[16] = NTOT - sum_p[:16].sum()
    n_mask = NTOT

    f1_list = np.asarray(f1_list, dtype=np.float64)
    has = count > 0
    pm = sum_p > 0
    precision = np.where(pm, nom / np.where(pm, sum_p, 1.0), 0.0)
    recall = np.where(has, nom / np.where(has, count, 1.0), 0.0)
    neg = n_mask - count
    spec_num = (n_mask - sum_p) - (count - nom)
    nmp = neg > 0
    specificity = np.where(nmp, spec_num / np.where(nmp, neg, 1.0), 0.0)

    def bce(x):
        return np.minimum(-np.log(np.maximum(x, 1e-38)), 100.0)

    loss_list = np.where(
        has,
        np.where(pm, bce(precision), 0.0) + bce(recall)
        + np.where(nmp, bce(specificity), 0.0),
        0.0)

    denom = precision + recall
    f1 = np.where(denom > 0, 2.0 * precision * recall
                  / np.where(denom > 0, denom, 1.0), 0.0)
    cur_f1 = np.where(has, f1, 0.0)
    new_f1 = BETA * f1_list + (1.0 - BETA) * cur_f1

    cnt = has.sum()
    sel = loss_list != 0
    logits = np.where(sel, ALPHA * (1.0 - new_f1), -np.inf)
    mx = logits.max()
    ex = np.exp(logits - mx)
    sm = ex / ex.sum()
    weighted = loss_list * (1.0 + WPC * cnt * sm)
    loss = weighted.sum() / (cnt * (1.0 + WPC))
    return np.float32(loss)


def kernel(pred, ssc_target, f1_list):
    nc = _get_nc()
    in_maps = _shard_inputs(pred, ssc_target)
    res = run_bass_kernel_spmd(nc, in_maps, core_ids=list(range(N_CORES)))
    outs = [np.asarray(r["out"], dtype=np.float32) for r in res.results]
    return _postprocess(outs, f1_list).reshape(())


if __name__ == "__main__":
    rng = np.random.default_rng(0)
    pred = rng.standard_normal((2, C, 200, 200, 16), dtype=np.float32)
    tgt = rng.integers(0, C, size=(2, 200, 200, 16)).astype(np.int64)
    f1l = np.zeros((C,), np.float32)
    print(kernel(pred, tgt, f1l))


# revision 12
# speedup vs baseline: 1.2212x; 1.0220x over previous
"""Trainium2 Bass kernel for the adaptive semantic-scal loss (segment_reduce).

Self-contained: hardcodes shapes/sharding for
  pred [2,17,200,200,16] f32, ssc_target [2,200,200,16] int, f1_list [17] f32.

Strategy (8 NeuronCores, data-parallel over voxels; 160k voxels/core laid
out as 128 partitions x 1250 voxels, slab-major / class-major within each
partition; every 125-voxel chunk gets a leading "gap" column):

  device, per core: pipelined slabs (3+2+2+2+1 chunks of 126 cols).
    pred ships as fp8_e4m3 (halves HBM traffic; noise averages out in the
    75k-element class sums).
    ACT: E = exp(pred) per slab (the hard floor: ~19us at 1 elem/cyc).
    DVE: onehot prebuilt for the whole tile (overlaps the DMA ramp),
         per slab: class-tree-sum -> S, fast reciprocal -> W (bf16),
         R = E*W in place, gap columns of R set to 1.
    PE:  per class c<16, per chunk: psum += OH_chunk^T @ R_chunk; classes
         c and c+8 share PSUM bank c as ONE accumulation chain (a start
         clears the whole bank's has_written bits, so only one start per
         bank; the sibling's first write lands on cleared bits and
         overwrites). Gap columns make row 0 = sum_p partials and col 0 =
         count partials; the diagonal holds nominator partials.
    class 16 nominator: one fused STT (onehot*R + free-dim accum) per slab,
         written straight into the output tile.
    extraction (once): mask-mult + X-reduce (bf16, 4x mode) of the PSUM
         set -> per-position nominator/sum_p partials; strided copy of
         col 0 -> count partials.
  device output: [128, 64] f32 of partial sums per core - NO collective,
  NO on-device epilogue.

  host: gather 8x[128,64], sum partials (cores+partitions), close classes
  16/sum_p/count via softmax identities, run the 17-element scalar loss
  epilogue in numpy.
"""

import sys

for _p in ("/opt/trn_rl_repo",):
    if _p not in sys.path:
        sys.path.append(_p)

import numpy as np
import ml_dtypes

import concourse.bacc as bacc
import concourse.tile as tile
import concourse.mybir as mybir
from concourse.bass_utils import run_bass_kernel_spmd

F32 = mybir.dt.float32
BF16 = mybir.dt.bfloat16
FP8 = mybir.dt.float8e4
ALU = mybir.AluOpType
ACTF = mybir.ActivationFunctionType

N_CORES = 8
P = 128          # partitions
C = 17           # classes
KV = 1250        # real voxels per partition per core (128*1250*8 = 1.28M)
W = 125          # data voxels per matmul chunk
WP = W + 1       # chunk width incl. leading ones-gap column
NCH = 10         # chunks per partition
KVP = NCH * WP   # padded voxels per partition (1260)
CPSL = [1, 2, 3, 3, 1]            # chunks per slab
NSLAB = len(CPSL)
COFF = [sum(CPSL[:i]) for i in range(NSLAB)]   # chunk offset per slab

BETA = 0.95
ALPHA = 5.0
WPC = 3.0
NTOT = float(N_CORES * P * KV)  # all targets are valid (0..16)


def _build():
    nc = bacc.Bacc("TRN2", target_bir_lowering=False, debug=False,
                   num_devices=N_CORES)
    pred_d = nc.dram_tensor("pred", [P, C * KVP], FP8, kind="ExternalInput")
    tgt_d = nc.dram_tensor("tgt", [P, KVP], BF16, kind="ExternalInput")
    out_d = nc.dram_tensor("out", [P, 64], F32, kind="ExternalOutput")

    # extraction mask over a [126, 256] two-class bank view: per half,
    # diag (1..125) -> nominator cells, row 0 (cols>=1) -> sum_p cells;
    # [*, half*128] stays 0 (gap x gap junk)
    m2 = np.zeros((128, 256), np.float32)
    for half in range(2):
        for k in range(1, WP):
            m2[k, half * 128 + k] = 1.0
            m2[0, half * 128 + k] = 1.0

    with tile.TileContext(nc) as tc:
        with (
            tc.tile_pool(name="pred", bufs=1) as pk,
            tc.tile_pool(name="work", bufs=1) as pw,
            tc.tile_pool(name="small", bufs=2) as ps,
            tc.tile_pool(name="persist", bufs=1) as pa,
            tc.tile_pool(name="psum", bufs=1, space="PSUM") as pp,
        ):
            tgt_sb = pa.tile([P, KVP], BF16)
            nc.sync.dma_start(out=tgt_sb[:, :], in_=tgt_d[:, :])

            pred_sb = pk.tile([P, C, KVP], FP8)   # slab-major runs
            for s in range(NSLAB):
                a = C * WP * COFF[s]
                b = C * WP * (COFF[s] + CPSL[s])
                nc.sync.dma_start(
                    out=pred_sb[:, :, :].rearrange("p c k -> p (c k)")[:, a:b],
                    in_=pred_d[:, a:b])

            mask2_d = nc.inline_tensor(m2.astype(ml_dtypes.bfloat16),
                                       name="mask2")
            mask2 = pa.tile([128, 256], BF16)
            nc.sync.dma_start(out=mask2[:, :], in_=mask2_d[:, :])

            # slab views: [P, C, w] starting at slab-major flat offsets
            def slab_view(tile_, s, dt_w=1):
                flat = tile_[:, :, :].rearrange("p c k -> p (c k)")
                a = C * WP * COFF[s]
                w = WP * CPSL[s]
                return flat[:, a:a + C * w].rearrange(
                    "p (c k) -> p c k", c=C)

            ER = pw.tile([P, C, KVP], BF16)        # E, then R in place
            OH = pa.tile([P, 16, KVP], BF16)       # onehot, chunk-major
            out_sb = pa.tile([P, 64], F32)

            # ---- ACT: exp per slab ----------------------------------
            for s in range(NSLAB):
                a = C * WP * COFF[s]
                b = C * WP * (COFF[s] + CPSL[s])
                nc.scalar.activation(
                    ER[:, :, :].rearrange("p c k -> p (c k)")[:, a:b],
                    pred_sb[:, :, :].rearrange("p c k -> p (c k)")[:, a:b],
                    ACTF.Exp)

            # ---- DVE queue ------------------------------------------
            # onehot prebuild for the whole tile (depends only on tgt,
            # which lands first -> overlaps the pred DMA / ACT ramp)
            for c in range(16):
                nc.vector.tensor_scalar(OH[:, c, :], tgt_sb[:, :],
                                        float(c), None, ALU.is_equal)
            # gap columns of OH -> 1 (row-0 sum_p trick)
            nc.vector.memset(
                OH[:, :, :].rearrange("p c (g k) -> p c g k",
                                      g=NCH)[:, :, :, 0], 1.0)

            def emit_slab_dve(s):
                w = WP * CPSL[s]
                T8 = ps.tile([P, 8, WP * 3], BF16, name="t8_%d" % s,
                             tag="t8", bufs=2)
                S = ps.tile([P, WP * 3], F32, name="s_%d" % s, tag="s",
                            bufs=2)
                Wf = ps.tile([P, WP * 3], F32, name="wf_%d" % s, tag="wf",
                             bufs=2)
                Wb = ps.tile([P, WP * 3], BF16, name="w_%d" % s, tag="w",
                             bufs=2)
                dmp = ps.tile([P, WP * 3], BF16, name="d_%d" % s, tag="d",
                              bufs=2)
                e = slab_view(ER, s)
                nc.vector.tensor_add(T8[:, :, 0:w], e[:, 0:8, :],
                                     e[:, 8:16, :])
                nc.vector.tensor_add(T8[:, 0:4, 0:w], T8[:, 0:4, 0:w],
                                     T8[:, 4:8, 0:w])
                nc.vector.tensor_add(T8[:, 0:2, 0:w], T8[:, 0:2, 0:w],
                                     T8[:, 2:4, 0:w])
                nc.vector.tensor_add(T8[:, 0, 0:w], T8[:, 0, 0:w],
                                     T8[:, 1, 0:w])
                nc.vector.tensor_add(S[:, 0:w], T8[:, 0, 0:w], e[:, 16, :])
                nc.vector.reciprocal_approx_fast(Wf[:, 0:w], S[:, 0:w])
                nc.vector.tensor_copy(Wb[:, 0:w], Wf[:, 0:w])
                wb = Wb[:, 0:w].rearrange("p (a k) -> p a k", a=1) \
                    .to_broadcast((P, C, w))
                nc.vector.tensor_tensor(e[:, :, :], e[:, :, :], wb,
                                        op=ALU.mult)
                # gap columns of R -> 1 (col-0 count trick), classes 0..15
                nc.vector.memset(
                    e[:, 0:16, :].rearrange("p c (g k) -> p c g k",
                                            g=CPSL[s])[:, :, :, 0], 1.0)
                # class-16 nominator partial straight into the output tile
                tg = tgt_sb[:, WP * COFF[s]:WP * COFF[s] + w]
                nc.vector.scalar_tensor_tensor(
                    out=dmp[:, 0:w], in0=tg, scalar=16.0,
                    in1=e[:, 16, :], op0=ALU.is_equal, op1=ALU.mult,
                    accum_out=out_sb[:, 32 + s:33 + s])

            for s in range(NSLAB):
                emit_slab_dve(s)

            # ---- PE: 8 banks, one accumulation chain per bank -------
            pnom = pp.tile([128, 8, 512], F32)
            for s in range(NSLAB):
                e = slab_view(ER, s)
                for h in range(CPSL[s]):
                    g = COFF[s] + h
                    for c in range(16):
                        bank, colb = c % 8, 128 * (c // 8)
                        nc.tensor.matmul(
                            pnom[0:WP, bank, colb:colb + WP],
                            OH[:, c, g * WP:(g + 1) * WP],
                            e[:, c, h * WP:(h + 1) * WP],
                            start=(g == 0 and c < 8),
                            stop=(g == NCH - 1 and c >= 8),
                            skip_group_check=True)

            # ---- extraction (once) ----------------------------------
            # device class order in the 16 output cols: j = 2*bank + half,
            # i.e. class = (j%2)*8 + j//2 (host unpermutes)
            nd = pw.tile([128, 8, 256], BF16)
            m2b = mask2[0:WP, :].rearrange("p (a k) -> p a k", a=1) \
                .to_broadcast((WP, 8, 256))
            nc.vector.tensor_tensor(nd[0:WP, :, :], pnom[0:WP, :, 0:256],
                                    m2b, op=ALU.mult)
            # rows 1..125 -> nominator partials, row 0 -> sum_p partials
            nc.vector.tensor_reduce(
                out_sb[0:WP, 16:32],
                nd[0:WP, :, :].rearrange("p b (h k) -> p (b h) k", h=2),
                axis=mybir.AxisListType.X, op=ALU.add)
            # count partials: col 0 of each half-region (row 0 junk,
            # host skips it)
            nc.vector.tensor_copy(
                out_sb[0:WP, 0:16],
                pnom[0:WP, :, 0:256].rearrange("p b (h k) -> p b h k",
                                               h=2)[:, :, :, 0])
            nc.sync.dma_start(out=out_d[:, :], in_=out_sb[:, :])

    nc.compile()
    return nc


_NC_CACHE = None


def _get_nc():
    global _NC_CACHE
    if _NC_CACHE is None:
        _NC_CACHE = _build()
    return _NC_CACHE


def _shard_inputs(pred, ssc_target, f1_list=None):
    pred = np.asarray(pred, dtype=np.float32)
    tgt = np.asarray(ssc_target)

    nvox = N_CORES * P * KV
    assert nvox == pred.size // C
    # voxel-major [v, c], then block: [core, p, c, kv]
    pv = np.ascontiguousarray(
        pred.reshape(2, C, -1).transpose(0, 2, 1).reshape(nvox, C)
        .reshape(N_CORES, P, KV, C).transpose(0, 1, 3, 2))
    tv = tgt.reshape(nvox).reshape(N_CORES, P, KV)
    # pad: each 125-voxel chunk gets a leading gap column
    # (pred=0 -> E=1; tgt=255 -> onehot=0)
    pp_ = np.zeros((N_CORES, P, C, NCH, WP), np.float32)
    pp_[..., 1:] = pv.reshape(N_CORES, P, C, NCH, W)
    # slab-major layout: [core, p, slab, c, slab cols]
    pp_ = pp_.reshape(N_CORES, P, C, KVP)
    parts = []
    for s in range(NSLAB):
        a = WP * COFF[s]
        b = a + WP * CPSL[s]
        parts.append(pp_[:, :, :, a:b].reshape(N_CORES, P, -1))
    pf = np.ascontiguousarray(np.concatenate(parts, axis=2)) \
        .astype(ml_dtypes.float8_e4m3)
    tp = np.full((N_CORES, P, NCH, WP), 255.0, np.float32)
    tp[..., 1:] = tv.reshape(N_CORES, P, NCH, W)
    tp = tp.reshape(N_CORES, P, KVP).astype(ml_dtypes.bfloat16)
    return [{"pred": pf[i], "tgt": tp[i]} for i in range(N_CORES)]


def _postprocess(outs, f1_list):
    """outs: list of per-core [128, 64] f32 partial tiles -> scalar loss."""
    a = np.asarray(outs, dtype=np.float64)          # [cores, 128, 64]
    count = np.zeros(C)
    sum_p = np.zeros(C)
    nom = np.zeros(C)
    # device col j = 2*(c%8) + (c//8) for class c
    perm = np.array([2 * (c % 8) + (c // 8) for c in range(16)])
    count[:16] = a[:, 1:WP, 0:16].sum(axis=(0, 1))[perm]
    nom[:16] = a[:, 1:WP, 16:32].sum(axis=(0, 1))[perm]
    sum_p[:16] = a[:, 0, 16:32].sum(axis=0)[perm]
    nom[16] = a[:, :, 32:32 + NSLAB].sum()
    count[16] = NTOT - count[:16].sum()
    sum_p[16] = NTOT - sum_p[:16].sum()
    n_mask = NTOT

    f1_list = np.asarray(f1_list, dtype=np.float64)
    has = count > 0
    pm = sum_p > 0
    precision = np.where(pm, nom / np.where(pm, sum_p, 1.0), 0.0)
    recall = np.where(has, nom / np.where(has, count, 1.0), 0.0)
    neg = n_mask - count
    spec_num = (n_mask - sum_p) - (count - nom)
    nmp = neg > 0
    specificity = np.where(nmp, spec_num / np.where(nmp, neg, 1.0), 0.0)

    def bce(x):
        return np.minimum(-np.log(np.maximum(x, 1e-38)), 100.0)

    loss_list = np.where(
        has,
        np.where(pm, bce(precision), 0.0) + bce(recall)
        + np.where(nmp, bce(specificity), 0.0),
        0.0)

    denom = precision + recall
    f1 = np.where(denom > 0, 2.0 * precision * recall
                  / np.where(denom > 0, denom, 1.0), 0.0)
    cur_f1 = np.where(has, f1, 0.0)
    new_f1 = BETA * f1_list + (1.0 - BETA) * cur_f1

    cnt = has.sum()
    sel = loss_list != 0
    logits = np.where(sel, ALPHA * (1.0 - new_f1), -np.inf)
    mx = logits.max()
    ex = np.exp(logits - mx)
    sm = ex / ex.sum()
    weighted = loss_list * (1.0 + WPC * cnt * sm)
    loss = weighted.sum() / (cnt * (1.0 + WPC))
    return np.float32(loss)


def kernel(pred, ssc_target, f1_list):
    nc = _get_nc()
    in_maps = _shard_inputs(pred, ssc_target)
    res = run_bass_kernel_spmd(nc, in_maps, core_ids=list(range(N_CORES)))
    outs = [np.asarray(r["out"], dtype=np.float32) for r in res.results]
    return _postprocess(outs, f1_list).reshape(())


if __name__ == "__main__":
    rng = np.random.default_rng(0)
    pred = rng.standard_normal((2, C, 200, 200, 16), dtype=np.float32)
    tgt = rng.integers(0, C, size=(2, 200, 200, 16)).astype(np.int64)
    f1l = np.zeros((C,), np.float32)
    print(kernel(pred, tgt, f1l))
